# revision 1
# baseline (speedup 1.0000x reference)
"""Trainium2 Bass kernel for ConvTemporalGraphical-style gated graph conv.

Computation (see reference):
    g   = x.reshape(N, F)                       # F = C*T*V = 204800
    h0  = elu(g @ W0 + b0)                      # [N, 256]   <-- dominant cost
    h1  = elu(h0 @ W1 + b1)                     # [N, 256]
    w   = softmax(h1 @ W2 + b2)                 # [N, 4]
    AS  = einsum('ne,etvw->ntvw', w, A)         # [N, T, V, V]
    out = einsum('nctv,ntvw->nctw', x, AS)

Sharding across 8 NeuronCores (one chip):
  * The F (contraction) dim of the big gating matmul is split 8 ways: core c
    holds W0 rows [c*25600, (c+1)*25600) (26 MB instead of 210 MB) and the
    matching slice of x, producing a partial h0 [32, 256].
  * A tiny AllReduce (32 KB) combines the partials; every core then runs the
    small MLP + softmax redundantly for all 32 samples.
  * The mixture + graph conv is data-parallel: core c owns samples
    [4c, 4c+4), selected on-device via a per-core one-hot matrix so all
    cores run the same graph (SPMD).

Device-friendly input layouts are produced on the host while sharding:
  * xgT: the gating x slice pre-transposed to [128, 200, 32] bf16 k-chunks
    (contraction dim on partitions), so no on-device transposes are needed.
  * W0s: bf16 [25600, 256]; the 128-row k-chunks are DMA'd directly as
    matmul moving operands.  bf16 halves HBM traffic for the dominant
    tensor; fp32 PSUM accumulation keeps the end-to-end error ~4e-4.
  * xcT / A4p: conv-side tensors pre-arranged into a v-padded layout
    (partition = 32*b + v with t = 32*b + g) so the 25x25(x64) graph-conv
    matmuls can be packed 4-at-a-time into the PE array via `tile_position`
    row groups, with samples paired on PSUM partition halves (col groups)
    so output DMAs use all 128 partitions.
"""

import sys

if "/opt/trn_rl_repo" not in sys.path:
    sys.path.insert(0, "/opt/trn_rl_repo")

import numpy as np

import concourse.bass as bass
import concourse.mybir as mybir
import concourse.tile as tile
from concourse import bacc
from concourse import bass_utils
from concourse.masks import make_identity

# Problem dims (hardcoded per contract).
N, C, T, V = 32, 64, 128, 25
F = C * T * V            # 204800
H = 256
E = 4
NCORES = 8
KS = F // NCORES         # 25600 rows of W0 per core
NLOC = N // NCORES       # 4 samples per core (conv slice)
KCH = KS // 128          # 200 k-chunks of 128 per core
TG = T // 4              # 32 t-groups; t = 32*b + g (b = row block, g = group)
W0GRP = 10               # k-chunks per W0 load

FP32 = mybir.dt.float32
BF16 = mybir.dt.bfloat16
AX = mybir.AxisListType
ALU = mybir.AluOpType
ACTF = mybir.ActivationFunctionType

CFG = {
    "gating_dtype": "bf16",   # "bf16" | "f32"
    "conv_dtype": "f32",      # "bf16" | "f32"; conv feeds the output directly.
                              # f32: rel err 4.3e-4, ~106us model span.
                              # bf16: rel err 4.3e-3, ~92us model span.
    "conv_pair_cols": True,   # odd samples on PSUM col group 64
    "phase": 5,
}


def _gdt():
    return BF16 if CFG["gating_dtype"] == "bf16" else FP32


def _cdt():
    return BF16 if CFG["conv_dtype"] == "bf16" else FP32


def build():
    nc = bacc.Bacc("TRN2", target_bir_lowering=False, debug=False, num_devices=NCORES)

    gdt = _gdt()
    xgT = nc.dram_tensor("xgT", [128, KCH, N], gdt, kind="ExternalInput")
    xcT = nc.dram_tensor("xcT", [128, NLOC // 2, TG, 2 * C], _cdt(),
                         kind="ExternalInput")
    W0s = nc.dram_tensor("W0s", [KS, H], gdt, kind="ExternalInput")
    b0 = nc.dram_tensor("b0", [H], FP32, kind="ExternalInput")
    W1 = nc.dram_tensor("W1", [H, H], FP32, kind="ExternalInput")
    b1 = nc.dram_tensor("b1", [H], FP32, kind="ExternalInput")
    W2 = nc.dram_tensor("W2", [H, E], FP32, kind="ExternalInput")
    b2 = nc.dram_tensor("b2", [E], FP32, kind="ExternalInput")
    A4p = nc.dram_tensor("A4p", [128, E, TG * V], _cdt(), kind="ExternalInput")
    selT = nc.dram_tensor("selT", [N, NLOC], FP32, kind="ExternalInput")
    out = nc.dram_tensor("out", [NLOC, C, T * V], FP32, kind="ExternalOutput")

    with tile.TileContext(nc) as tc:
        _build_body(nc, tc, xgT, xcT, W0s, b0, W1, b1, W2, b2, A4p, selT, out)
    nc.compile()
    return nc


def _build_body(nc, tc, xgT, xcT, W0s, b0, W1, b1, W2, b2, A4p, selT, out):
    from contextlib import ExitStack

    def _as_ap(t):
        return t if isinstance(t, bass.AP) else t.ap()

    xgT, xcT, W0s, b0, W1, b1, W2, b2, A4p, selT, out = map(
        _as_ap, (xgT, xcT, W0s, b0, W1, b1, W2, b2, A4p, selT, out)
    )
    gdt = _gdt()
    cdt = _cdt()

    ctx = ExitStack()
    with ctx:
        const = ctx.enter_context(tc.tile_pool(name="const", bufs=1))
        w0_pool = ctx.enter_context(tc.tile_pool(name="w0_pool", bufs=10))
        mix_pool = ctx.enter_context(tc.tile_pool(name="mix_pool", bufs=2))
        out_pool = ctx.enter_context(tc.tile_pool(name="out_pool", bufs=2))
        dram = ctx.enter_context(tc.tile_pool(name="dram", bufs=1, space="DRAM"))
        # PSUM bank budget (8): pg 1 + ph 1 + pc 6 (po0/po1 double-buffered)
        pg = ctx.enter_context(tc.tile_pool(name="pg", bufs=1, space="PSUM"))
        ph = ctx.enter_context(tc.tile_pool(name="ph", bufs=1, space="PSUM"))
        pc = ctx.enter_context(tc.tile_pool(name="pc", bufs=1, space="PSUM"))

        # ---- persistent big SBUF tensors ----
        xT_all = const.tile([128, KCH, N], gdt)           # gating x^T chunks
        xcT_all = const.tile([128, NLOC // 2, TG, 2 * C], cdt)
        A_sb = const.tile([128, E, TG * V], cdt)          # padded A
        AS_sb = const.tile([128, NLOC, TG * V], cdt)      # mixture output

        # ---- bulk input loads (pre-transposed / pre-padded on host) ----
        Q = KCH // 4
        for q in range(4):
            eng = nc.sync if q % 2 == 0 else nc.scalar
            eng.dma_start(xT_all[:, q * Q:(q + 1) * Q, :], xgT[:, q * Q:(q + 1) * Q, :])

        # =========================================================
        # Gating matmul: 200-chunk fp32-accumulated bf16 matmuls
        # =========================================================
        h0_ps = pg.tile([N, H], FP32)
        for g in range(KCH // W0GRP):
            w0_t = w0_pool.tile([128, W0GRP, H], gdt, tag="w0_t")
            w0_src = W0s.rearrange("(g j p) h -> g p j h", j=W0GRP, p=128)[g]
            dma_eng = nc.sync if g % 2 == 0 else nc.scalar
            dma_eng.dma_start(w0_t[:], w0_src)
            for j in range(W0GRP):
                k = g * W0GRP + j
                nc.tensor.matmul(
                    h0_ps[:],
                    xT_all[:, k, :],
                    w0_t[:, j, :],
                    start=(k == 0),
                    stop=(k == KCH - 1),
                )

        # constants + conv-side loads, emitted late so they fill DMA gaps /
        # the collective wait rather than delaying the W0 stream.
        identity = const.tile([128, 128], FP32)
        make_identity(nc, identity)

        b0_row = const.tile([1, H], FP32)
        nc.sync.dma_start(b0_row[:], b0.rearrange("(o h) -> o h", o=1))
        b0b = const.tile([N, H], FP32)
        nc.gpsimd.partition_broadcast(b0b[:], b0_row[:])

        b1_row = const.tile([1, H], FP32)
        nc.scalar.dma_start(b1_row[:], b1.rearrange("(o h) -> o h", o=1))
        b1b = const.tile([N, H], FP32)
        nc.gpsimd.partition_broadcast(b1b[:], b1_row[:])

        b2_row = const.tile([1, E], FP32)
        nc.sync.dma_start(b2_row[:], b2.rearrange("(o h) -> o h", o=1))
        b2b = const.tile([N, E], FP32)
        nc.gpsimd.partition_broadcast(b2b[:], b2_row[:])

        W1_sb = const.tile([128, 2, H], FP32)
        nc.scalar.dma_start(W1_sb[:], W1.rearrange("(j p) h -> p j h", p=128))
        W2_sb = const.tile([128, 2, E], FP32)
        nc.sync.dma_start(W2_sb[:], W2.rearrange("(j p) h -> p j h", p=128))
        selT_sb = const.tile([N, NLOC], FP32)
        nc.scalar.dma_start(selT_sb[:], selT[:])

        if CFG["phase"] == 1:
            p1 = const.tile([N, H], FP32)
            nc.vector.tensor_copy(p1[:], h0_ps[:])
            nc.sync.dma_start(out[0][:N, :H], p1[:])
            return

        # =========================================================
        # Partial-h0 AllReduce (tiny; runs on TOPSP/SDMA silicon)
        # =========================================================
        h0p_sb = const.tile([N, H], FP32)
        nc.vector.tensor_copy(h0p_sb[:], h0_ps[:])
        cc_in = dram.tile([N, H], FP32)
        cc_out = dram.tile([N, H], FP32, addr_space="Shared")
        nc.sync.dma_start(cc_in[:], h0p_sb[:])
        nc.gpsimd.collective_compute(
            "AllReduce",
            ALU.add,
            replica_groups=[list(range(NCORES))],
            ins=[cc_in.opt()],
            outs=[cc_out.opt()],
        )
        # conv-side loads land in the collective's idle window
        if CFG["phase"] >= 4:
            nc.sync.dma_start(xcT_all[:, 0], xcT[:, 0])
            nc.scalar.dma_start(xcT_all[:, 1], xcT[:, 1])
            nc.sync.dma_start(A_sb[:], A4p[:])

        h0_sb = const.tile([N, H], FP32)
        nc.sync.dma_start(h0_sb[:], cc_out[:])
        if CFG["phase"] == 2:
            nc.sync.dma_start(out[0][:N, :H], h0_sb[:])
            return

        # =========================================================
        # Tiny MLP + softmax + local-w selection/broadcast
        # =========================================================
        def elu_inplace(t, width):
            tmp = const.tile([N, width], FP32, tag="elu_tmp", name="elu_tmp")
            nc.vector.tensor_scalar(tmp[:], t[:], 0.0, None, ALU.min)
            nc.scalar.activation(tmp[:], tmp[:], ACTF.Exp)
            nc.vector.tensor_scalar(t[:], t[:], 0.0, -1.0, ALU.max, ALU.add)
            nc.vector.tensor_tensor(t[:], t[:], tmp[:], ALU.add)

        nc.vector.tensor_tensor(h0_sb[:], h0_sb[:], b0b[:], ALU.add)
        elu_inplace(h0_sb, H)

        ps_h = ph.tile([128, 2 * N], FP32, tag="mlp_ps")
        for j in range(2):
            nc.tensor.transpose(
                ps_h[:, j * N:(j + 1) * N],
                h0_sb[:, j * 128:(j + 1) * 128],
                identity[:N, :N],
            )
        h0T = const.tile([128, 2, N], FP32)
        nc.vector.tensor_copy(h0T[:].rearrange("p j n -> p (j n)"), ps_h[:])

        h1_ps = ph.tile([N, H], FP32, tag="mlp_ps")
        for j in range(2):
            nc.tensor.matmul(
                h1_ps[:], h0T[:, j, :], W1_sb[:, j, :],
                start=(j == 0), stop=(j == 1),
            )
        h1_sb = const.tile([N, H], FP32)
        nc.vector.tensor_copy(h1_sb[:], h1_ps[:])
        nc.vector.tensor_tensor(h1_sb[:], h1_sb[:], b1b[:], ALU.add)
        elu_inplace(h1_sb, H)

        ps_h2 = ph.tile([128, 2 * N], FP32, tag="mlp_ps")
        for j in range(2):
            nc.tensor.transpose(
                ps_h2[:, j * N:(j + 1) * N],
                h1_sb[:, j * 128:(j + 1) * 128],
                identity[:N, :N],
            )
        h1T = const.tile([128, 2, N], FP32)
        nc.vector.tensor_copy(h1T[:].rearrange("p j n -> p (j n)"), ps_h2[:])

        lg_ps = ph.tile([N, E], FP32, tag="mlp_ps")
        for j in range(2):
            nc.tensor.matmul(
                lg_ps[:], h1T[:, j, :], W2_sb[:, j, :],
                start=(j == 0), stop=(j == 1),
            )
        lg_sb = const.tile([N, E], FP32)
        nc.vector.tensor_copy(lg_sb[:], lg_ps[:])
        nc.vector.tensor_tensor(lg_sb[:], lg_sb[:], b2b[:], ALU.add)

        # softmax over E (free dim); logits are bounded (|x| < ~2: elu-bounded
        # h1 times U(+-1/16) weights over K=256), so skip the max-subtraction.
        ex = const.tile([N, E], FP32)
        sm = const.tile([N, 1], FP32)
        nc.scalar.activation(ex[:], lg_sb[:], ACTF.Exp, accum_out=sm[:])
        rec = const.tile([N, 1], FP32)
        nc.vector.reciprocal(rec[:], sm[:])
        w_sb = const.tile([N, E], FP32)
        nc.vector.tensor_scalar(w_sb[:], ex[:], rec[:], None, ALU.mult)

        # local w: [4, 4] = selT^T @ w  (K = 32 on partitions)
        wl_ps = ph.tile([NLOC, E], FP32, tag="mlp_ps")
        nc.tensor.matmul(wl_ps[:], selT_sb[:], w_sb[:], start=True, stop=True)
        wloc = const.tile([NLOC, E], FP32)
        nc.vector.tensor_copy(wloc[:], wl_ps[:])

        # flatten [4, 4] -> [1, 16] (partition-crossing SBUF DMA), broadcast.
        w_row = const.tile([1, NLOC * E], FP32)
        nc.gpsimd.dma_start(
            w_row.rearrange("o (n e) -> o n e", n=NLOC), wloc[:]
        )
        w_bcast = const.tile([128, NLOC * E], FP32)
        nc.gpsimd.partition_broadcast(w_bcast[:], w_row[:])
        if CFG["phase"] == 3:
            nc.sync.dma_start(out[0][:, :NLOC * E], w_bcast[:C, :])
            return

        # =========================================================
        # Mixture AS[n] = sum_e w[n,e] * A[e] interleaved with the graph
        # conv per sample-pair, so conv pair 0 starts as soon as AS[0..1]
        # are ready instead of after all four mixtures.
        # =========================================================
        def emit_mixture(n):
            # adds are the DVE-bound part; odd samples' adds run on GpSimd
            # (idle in this window) to halve the mixture wall time.
            add_eng = nc.vector if n % 2 == 0 else nc.gpsimd
            acc = mix_pool.tile([128, TG * V], cdt, tag="mix_acc", name="acc")
            tmp = mix_pool.tile([128, TG * V], cdt, tag="mix_tmp", name="tmp")
            nc.scalar.activation(
                acc[:], A_sb[:, 0, :], ACTF.Copy, scale=w_bcast[:, n * E:n * E + 1]
            )
            nc.vector.tensor_scalar(
                tmp[:], A_sb[:, 1, :], w_bcast[:, n * E + 1:n * E + 2], None, ALU.mult
            )
            add_eng.tensor_tensor(acc[:], acc[:], tmp[:], ALU.add)
            nc.scalar.activation(
                tmp[:], A_sb[:, 2, :], ACTF.Copy, scale=w_bcast[:, n * E + 2:n * E + 3]
            )
            add_eng.tensor_tensor(acc[:], acc[:], tmp[:], ALU.add)
            nc.vector.tensor_scalar(
                tmp[:], A_sb[:, 3, :], w_bcast[:, n * E + 3:n * E + 4], None, ALU.mult
            )
            add_eng.tensor_tensor(AS_sb[:, n, :], acc[:], tmp[:], ALU.add)

        pair_cols = CFG["conv_pair_cols"]

        def emit_conv_pair(pr):
            ot = out_pool.tile([128, T * V], FP32, tag="ot", name="ot")
            for g0, glen in ((0, 20), (20, 12)):
                # width padded to 512 so the row stride is bank-aligned
                pob = [
                    pc.tile([128, 512], FP32, tag=f"po{b}", name=f"po{b}",
                            bufs=2 if b < 2 else 1)
                    for b in range(4)
                ]
                for gi in range(glen):
                    g = g0 + gi
                    for b in range(4):
                        for j in range(2):
                            n = 2 * pr + j
                            nc.tensor.matmul(
                                pob[b][64 * j:64 * (j + 1),
                                       gi * V:(gi + 1) * V],
                                xcT_all[32 * b:32 * b + V, pr, g,
                                        64 * j:64 * (j + 1)],
                                AS_sb[32 * b:32 * b + V, n, g * V:(g + 1) * V],
                                start=True,
                                stop=True,
                                tile_position=(32 * b, 64 * j if pair_cols else 0),
                            )
                width = glen * V
                for b in range(4):
                    dst = ot[:, (32 * b + g0) * V:(32 * b + g0) * V + width]
                    if b % 2 == 0:
                        nc.vector.tensor_copy(dst, pob[b][:, :width])
                    else:
                        nc.scalar.activation(dst, pob[b][:, :width], ACTF.Copy)
                dma_eng = nc.sync if pr % 2 == 0 else nc.scalar
                od = out[2 * pr:2 * pr + 2].rearrange("n c f -> (n c) f")
                dma_eng.dma_start(
                    od.rearrange("r (b q) -> r b q", b=4)[:, :, g0 * V:g0 * V + width],
                    ot.rearrange("r (b q) -> r b q", b=4)[:, :, g0 * V:g0 * V + width],
                )

        emit_mixture(0)
        emit_mixture(1)
        if CFG["phase"] == 4:
            emit_mixture(2)
            emit_mixture(3)
            nc.sync.dma_start(out[0][:, :TG * V], AS_sb[:C, 0, :])
            return
        emit_conv_pair(0)
        emit_mixture(2)
        emit_mixture(3)
        emit_conv_pair(1)


_NC_CACHE = {}


def _get_nc():
    key = (CFG["gating_dtype"], CFG["conv_dtype"], CFG["conv_pair_cols"], CFG["phase"])
    if key not in _NC_CACHE:
        _NC_CACHE[key] = build()
    return _NC_CACHE[key]


def _to_bf16(a):
    """Round-to-nearest-even fp32 -> bf16, vectorized."""
    import ml_dtypes

    u = np.ascontiguousarray(a, dtype=np.float32).view(np.uint32)
    r = ((u + 0x7FFF + ((u >> 16) & 1)) >> 16).astype(np.uint16)
    return r.view(ml_dtypes.bfloat16)


def _shard_inputs(x, W0, b0, W1, b1, W2, b2, A):
    x = np.ascontiguousarray(np.asarray(x, dtype=np.float32))
    W0 = np.ascontiguousarray(np.asarray(W0, dtype=np.float32))
    A = np.ascontiguousarray(np.asarray(A, dtype=np.float32)).reshape(E, T, V, V)
    xf = x.reshape(N, F)
    bf16 = CFG["gating_dtype"] == "bf16"
    cbf16 = CFG["conv_dtype"] == "bf16"

    # A in padded layout: A4p[32b+v, e, g*V+w] = A[e, 32b+g, v, w]
    A4p = np.zeros((128, E, TG * V), dtype=np.float32)
    At = A.reshape(E, 4, TG, V, V)            # e b g v w
    for b in range(4):
        A4p[32 * b:32 * b + V, :, :] = (
            At[:, b].transpose(2, 0, 1, 3).reshape(V, E, TG * V)
        )

    A4p_cast = _to_bf16(A4p) if cbf16 else A4p
    in_maps = []
    for c in range(NCORES):
        sel = np.zeros((N, NLOC), dtype=np.float32)
        for i in range(NLOC):
            sel[c * NLOC + i, i] = 1.0

        # gating slice, pre-transposed to [128, KCH, N]
        xg = xf[:, c * KS:(c + 1) * KS]                   # [N, KS]
        xgT = np.ascontiguousarray(
            xg.reshape(N, KCH, 128).transpose(2, 1, 0)    # [128, KCH, N]
        )
        if bf16:
            xgT = np.ascontiguousarray(_to_bf16(xgT))
            W0c = np.ascontiguousarray(_to_bf16(W0[c * KS:(c + 1) * KS]))
        else:
            W0c = np.ascontiguousarray(W0[c * KS:(c + 1) * KS])

        # conv slice, pre-transposed/padded:
        # xcT[32b+v, pr, g, 64j+cc] = x[4c + 2pr + j, cc, 32b+g, v]
        xl = x[c * NLOC:(c + 1) * NLOC]                   # [4, C, T, V]
        xcT = np.zeros((128, NLOC // 2, TG, 2 * C), dtype=np.float32)
        xr = xl.reshape(NLOC // 2, 2, C, 4, TG, V)        # pr j cc b g v
        for b in range(4):
            # [pr, j, cc, g, v] -> [v, pr, g, (j cc)]
            blk = xr[:, :, :, b]                          # pr j cc g v
            xcT[32 * b:32 * b + V] = (
                blk.transpose(4, 0, 3, 1, 2).reshape(V, NLOC // 2, TG, 2 * C)
            )

        in_maps.append({
            "xgT": xgT,
            "xcT": _to_bf16(xcT) if cbf16 else xcT,
            "W0s": W0c,
            "b0": np.asarray(b0, dtype=np.float32),
            "W1": np.asarray(W1, dtype=np.float32),
            "b1": np.asarray(b1, dtype=np.float32),
            "W2": np.asarray(W2, dtype=np.float32),
            "b2": np.asarray(b2, dtype=np.float32),
            "A4p": A4p_cast,
            "selT": sel,
        })
    return in_maps


def kernel(x, W0, b0, W1, b1, W2, b2, A):
    nc = _get_nc()
    in_maps = _shard_inputs(x, W0, b0, W1, b1, W2, b2, A)
    res = bass_utils.run_bass_kernel_spmd(nc, in_maps, core_ids=list(range(NCORES)))
    outs = [res.results[c]["out"].reshape(NLOC, C, T, V) for c in range(NCORES)]
    return np.concatenate(outs, axis=0)



# revision 3
# speedup vs baseline: 9.3978x; 9.3978x over previous
"""Trainium2 Bass kernel for ConvTemporalGraphical-style gated graph conv.

Computation (see reference):
    g   = x.reshape(N, F)                       # F = C*T*V = 204800
    h0  = elu(g @ W0 + b0)                      # [N, 256]   <-- dominant cost
    h1  = elu(h0 @ W1 + b1)                     # [N, 256]
    w   = softmax(h1 @ W2 + b2)                 # [N, 4]
    AS  = einsum('ne,etvw->ntvw', w, A)         # [N, T, V, V]
    out = einsum('nctv,ntvw->nctw', x, AS)

Sharding across 8 NeuronCores (one chip):
  * The F (contraction) dim of the big gating matmul is split 8 ways: core c
    holds W0 rows [c*25600, (c+1)*25600) and the matching slice of x,
    producing a partial h0 [32, 256].
  * A tiny ReduceScatter (4 KB out per core) both combines the partials and
    hands core c exactly the 4 samples it owns; the small MLP + softmax then
    runs on [4, 256] locally.
  * The mixture + graph conv is data-parallel: core c owns samples
    [4c, 4c+4).

Precision strategy (error budget is rel_err < 2e-2; this lands ~8e-3):
  * Gating operands in fp8 e4m3 (TRN variant, max 240) with power-of-2
    scaling (x*2^5, W0*2^15, fixed up by 2^-20 after the matmul), using
    DoubleRow perf mode for 2x PE throughput and half the HBM traffic of
    bf16. fp32 PSUM accumulation.
  * Conv-side tensors (x, A, AS, out) in bf16; fp32 PSUM for the conv
    matmuls. The f32 output is reconstructed on host from bf16.
  * The tiny MLP stays fp32.

Buffer packing: all per-core inputs ride in 3 DRAM tensors (one fp8 blob,
one bf16 blob, one f32 blob) + 1 bf16 output, which keeps the per-dispatch
PJRT buffer-handle count (and axon tunnel overhead) low.
"""

import sys

if "/opt/trn_rl_repo" not in sys.path:
    sys.path.insert(0, "/opt/trn_rl_repo")

import numpy as np

import concourse.bass as bass
import concourse.mybir as mybir
import concourse.tile as tile
from concourse import bacc
from concourse import bass_utils
from concourse.masks import make_identity

# Problem dims (hardcoded per contract).
N, C, T, V = 32, 64, 128, 25
F = C * T * V            # 204800
H = 256
E = 4
NCORES = 8
KS = F // NCORES         # 25600 rows of W0 per core
NLOC = N // NCORES       # 4 samples per core
KCH = KS // 128          # 200 k-chunks of 128 per core
TG = T // 4              # 32 t-groups; t = 32*b + g (b = row block, g = group)
W0GRP = 10               # k-chunks per W0 load

# fp8 scaling (power-of-2; undone after the gating matmul)
XSH = 5                  # x * 2^5   (|x| < 5.2  -> < 166, e4m3 max 240)
WSH = 15                 # W0 * 2^15 (|W0| < 2.3e-3 -> < 73)

FP32 = mybir.dt.float32
BF16 = mybir.dt.bfloat16
FP8 = mybir.dt.float8e4
AX = mybir.AxisListType
ALU = mybir.AluOpType
ACTF = mybir.ActivationFunctionType

# fp8 blob layout (elements)
SZ_XG = 128 * KCH * N            # 819200
SZ_W0 = KS * H                   # 6553600
# bf16 blob layout
SZ_XC = 128 * (NLOC // 2) * TG * (2 * C)   # 1048576
SZ_A4 = 128 * E * TG * V                   # 102400
# f32 blob layout: b0 | W1 | b1 | W2 | b2
OFF_B0 = 0
OFF_W1 = H
OFF_B1 = OFF_W1 + H * H
OFF_W2 = OFF_B1 + H
OFF_B2 = OFF_W2 + H * E
SZ_F32 = OFF_B2 + E

CFG = {
    "phase": 5,
    "rep": 1,
}


def build(rep=1, phase=5):
    nc = bacc.Bacc("TRN2", target_bir_lowering=False, debug=False, num_devices=NCORES)

    blob8 = nc.dram_tensor("blob8", [SZ_XG + SZ_W0], FP8, kind="ExternalInput")
    blobb = nc.dram_tensor("blobb", [SZ_XC + SZ_A4], BF16, kind="ExternalInput")
    blobf = nc.dram_tensor("blobf", [SZ_F32], FP32, kind="ExternalInput")
    out = nc.dram_tensor("out", [NLOC, C, T * V], BF16, kind="ExternalOutput")

    with tile.TileContext(nc) as tc:
        for _ in range(rep):
            _build_body(nc, tc, blob8, blobb, blobf, out, phase)
    nc.compile()
    return nc


def _build_body(nc, tc, blob8, blobb, blobf, out, phase):
    from contextlib import ExitStack

    def _as_ap(t):
        return t if isinstance(t, bass.AP) else t.ap()

    blob8, blobb, blobf, out = map(_as_ap, (blob8, blobb, blobf, out))

    xgT = blob8[:SZ_XG].rearrange("(p k n) -> p k n", p=128, k=KCH)
    W0s = blob8[SZ_XG:].rearrange("(r h) -> r h", h=H)
    xcT = blobb[:SZ_XC].rearrange("(p r g c) -> p r g c", p=128, r=NLOC // 2, g=TG)
    A4p = blobb[SZ_XC:].rearrange("(p e q) -> p e q", p=128, e=E)
    b0 = blobf[OFF_B0:OFF_B0 + H]
    W1 = blobf[OFF_W1:OFF_W1 + H * H].rearrange("(j p h) -> p j h", p=128, h=H)
    b1 = blobf[OFF_B1:OFF_B1 + H]
    W2 = blobf[OFF_W2:OFF_W2 + H * E].rearrange("(j p e) -> p j e", p=128, e=E)
    b2 = blobf[OFF_B2:OFF_B2 + E]

    ctx = ExitStack()
    with ctx:
        const = ctx.enter_context(tc.tile_pool(name="const", bufs=1))
        w0_pool = ctx.enter_context(tc.tile_pool(name="w0_pool", bufs=10))
        mix_pool = ctx.enter_context(tc.tile_pool(name="mix_pool", bufs=2))
        out_pool = ctx.enter_context(tc.tile_pool(name="out_pool", bufs=2))
        dram = ctx.enter_context(tc.tile_pool(name="dram", bufs=1, space="DRAM"))
        # PSUM bank budget (8): pg 1 + ph 1 + pc 6 (po0/po1 double-buffered)
        pg = ctx.enter_context(tc.tile_pool(name="pg", bufs=1, space="PSUM"))
        ph = ctx.enter_context(tc.tile_pool(name="ph", bufs=1, space="PSUM"))
        pc = ctx.enter_context(tc.tile_pool(name="pc", bufs=1, space="PSUM"))

        # ---- persistent big SBUF tensors ----
        xT_all = const.tile([128, KCH, N], FP8)           # gating x^T chunks
        xcT_all = const.tile([128, NLOC // 2, TG, 2 * C], BF16)
        A_sb = const.tile([128, E, TG * V], BF16)         # padded A
        AS_sb = const.tile([128, NLOC, TG * V], BF16)     # mixture output

        # ---- bulk input loads (pre-transposed / pre-padded on host) ----
        Q = KCH // 4
        for q in range(4):
            eng = nc.sync if q % 2 == 0 else nc.scalar
            eng.dma_start(xT_all[:, q * Q:(q + 1) * Q, :], xgT[:, q * Q:(q + 1) * Q, :])

        # =========================================================
        # Gating matmul: 100 fp8 DoubleRow matmuls (fp32 PSUM accum)
        # =========================================================
        h0_ps = pg.tile([N, H], FP32)
        for g in range(KCH // W0GRP):
            w0_t = w0_pool.tile([128, W0GRP, H], FP8, tag="w0_t")
            w0_src = W0s.rearrange("(g j p) h -> g p j h", j=W0GRP, p=128)[g]
            dma_eng = nc.sync if g % 2 == 0 else nc.scalar
            dma_eng.dma_start(w0_t[:], w0_src)
            for j in range(0, W0GRP, 2):
                k = g * W0GRP + j
                nc.tensor.matmul(
                    h0_ps[:],
                    xT_all[:, k:k + 2, :],
                    w0_t[:, j:j + 2, :],
                    start=(k == 0),
                    stop=(k == KCH - 2),
                    perf_mode=mybir.MatmulPerfMode.DoubleRow,
                )

        # constants + conv-side loads, emitted late so they fill DMA gaps /
        # the collective wait rather than delaying the W0 stream.
        identity = const.tile([128, 128], FP32)
        make_identity(nc, identity)

        b0_row = const.tile([1, H], FP32)
        nc.sync.dma_start(b0_row[:], b0.rearrange("(o h) -> o h", o=1))
        b0b = const.tile([NLOC, H], FP32)
        nc.gpsimd.partition_broadcast(b0b[:], b0_row[:])

        b1_row = const.tile([1, H], FP32)
        nc.scalar.dma_start(b1_row[:], b1.rearrange("(o h) -> o h", o=1))
        b1b = const.tile([NLOC, H], FP32)
        nc.gpsimd.partition_broadcast(b1b[:], b1_row[:])

        b2_row = const.tile([1, E], FP32)
        nc.sync.dma_start(b2_row[:], b2.rearrange("(o h) -> o h", o=1))
        b2b = const.tile([NLOC, E], FP32)
        nc.gpsimd.partition_broadcast(b2b[:], b2_row[:])

        W1_sb = const.tile([128, 2, H], FP32)
        nc.scalar.dma_start(W1_sb[:], W1)
        W2_sb = const.tile([128, 2, E], FP32)
        nc.sync.dma_start(W2_sb[:], W2)

        if phase == 1:
            p1 = const.tile([N, H], FP32)
            nc.vector.tensor_scalar(p1[:], h0_ps[:], 2.0 ** (-XSH - WSH), None, ALU.mult)
            ob = out.rearrange("n c f -> (n c) f")
            nc.sync.dma_start(ob[:N, :H], p1[:])
            return

        # =========================================================
        # Partial-h0 ReduceScatter: combines the 8 partials AND hands
        # core c its own 4 samples' rows (tiny; runs on TOPSP/SDMA).
        # The 2^-20 fp8 scale fixup rides on the PSUM->SBUF copy.
        # =========================================================
        h0p_sb = const.tile([N, H], FP32)
        nc.vector.tensor_scalar(h0p_sb[:], h0_ps[:], 2.0 ** (-XSH - WSH), None, ALU.mult)
        cc_in = dram.tile([N, H], FP32)
        cc_out = dram.tile([NLOC, H], FP32)
        nc.sync.dma_start(cc_in[:], h0p_sb[:])
        nc.gpsimd.collective_compute(
            "ReduceScatter",
            ALU.add,
            replica_groups=[list(range(NCORES))],
            ins=[cc_in.opt()],
            outs=[cc_out.opt()],
        )
        # conv-side loads land in the collective's idle window
        if phase >= 4:
            nc.sync.dma_start(xcT_all[:, 0], xcT[:, 0])
            nc.scalar.dma_start(xcT_all[:, 1], xcT[:, 1])
            nc.sync.dma_start(A_sb[:], A4p)

        h0_sb = const.tile([NLOC, H], FP32)
        nc.sync.dma_start(h0_sb[:], cc_out[:])
        if phase == 2:
            ob = out.rearrange("n c f -> (n c) f")
            nc.sync.dma_start(ob[:NLOC, :H], h0_sb[:])
            return

        # =========================================================
        # Tiny MLP + softmax on the 4 local samples
        # =========================================================
        def elu_inplace(t, width):
            tmp = const.tile([NLOC, width], FP32, tag="elu_tmp", name="elu_tmp")
            nc.vector.tensor_scalar(tmp[:], t[:], 0.0, None, ALU.min)
            nc.scalar.activation(tmp[:], tmp[:], ACTF.Exp)
            nc.vector.tensor_scalar(t[:], t[:], 0.0, -1.0, ALU.max, ALU.add)
            nc.vector.tensor_tensor(t[:], t[:], tmp[:], ALU.add)

        nc.vector.tensor_tensor(h0_sb[:], h0_sb[:], b0b[:], ALU.add)
        elu_inplace(h0_sb, H)

        ps_h = ph.tile([128, 2 * NLOC], FP32, tag="mlp_ps")
        for j in range(2):
            nc.tensor.transpose(
                ps_h[:, j * NLOC:(j + 1) * NLOC],
                h0_sb[:, j * 128:(j + 1) * 128],
                identity[:NLOC, :NLOC],
            )
        h0T = const.tile([128, 2, NLOC], FP32)
        nc.vector.tensor_copy(h0T[:].rearrange("p j n -> p (j n)"), ps_h[:])

        h1_ps = ph.tile([NLOC, H], FP32, tag="mlp_ps")
        for j in range(2):
            nc.tensor.matmul(
                h1_ps[:], h0T[:, j, :], W1_sb[:, j, :],
                start=(j == 0), stop=(j == 1),
            )
        h1_sb = const.tile([NLOC, H], FP32)
        nc.vector.tensor_copy(h1_sb[:], h1_ps[:])
        nc.vector.tensor_tensor(h1_sb[:], h1_sb[:], b1b[:], ALU.add)
        elu_inplace(h1_sb, H)

        ps_h2 = ph.tile([128, 2 * NLOC], FP32, tag="mlp_ps")
        for j in range(2):
            nc.tensor.transpose(
                ps_h2[:, j * NLOC:(j + 1) * NLOC],
                h1_sb[:, j * 128:(j + 1) * 128],
                identity[:NLOC, :NLOC],
            )
        h1T = const.tile([128, 2, NLOC], FP32)
        nc.vector.tensor_copy(h1T[:].rearrange("p j n -> p (j n)"), ps_h2[:])

        lg_ps = ph.tile([NLOC, E], FP32, tag="mlp_ps")
        for j in range(2):
            nc.tensor.matmul(
                lg_ps[:], h1T[:, j, :], W2_sb[:, j, :],
                start=(j == 0), stop=(j == 1),
            )
        lg_sb = const.tile([NLOC, E], FP32)
        nc.vector.tensor_copy(lg_sb[:], lg_ps[:])
        nc.vector.tensor_tensor(lg_sb[:], lg_sb[:], b2b[:], ALU.add)

        # softmax over E (free dim); logits are bounded (|x| < ~2: elu-bounded
        # h1 times U(+-1/16) weights over K=256), so skip the max-subtraction.
        ex = const.tile([NLOC, E], FP32)
        sm = const.tile([NLOC, 1], FP32)
        nc.scalar.activation(ex[:], lg_sb[:], ACTF.Exp, accum_out=sm[:])
        rec = const.tile([NLOC, 1], FP32)
        nc.vector.reciprocal(rec[:], sm[:])
        wloc = const.tile([NLOC, E], FP32)
        nc.vector.tensor_scalar(wloc[:], ex[:], rec[:], None, ALU.mult)

        # flatten [4, 4] -> [1, 16] (partition-crossing SBUF DMA), broadcast.
        w_row = const.tile([1, NLOC * E], FP32)
        nc.gpsimd.dma_start(
            w_row.rearrange("o (n e) -> o n e", n=NLOC), wloc[:]
        )
        w_bcast = const.tile([128, NLOC * E], FP32)
        nc.gpsimd.partition_broadcast(w_bcast[:], w_row[:])
        if phase == 3:
            ob = out.rearrange("n c f -> (n c) f")
            nc.sync.dma_start(ob[:C, :NLOC * E], w_bcast[:C, :])
            return

        # =========================================================
        # Mixture AS[n] = sum_e w[n,e] * A[e] interleaved with the graph
        # conv per sample-pair, so conv pair 0 starts as soon as AS[0..1]
        # are ready instead of after all four mixtures.
        # =========================================================
        def emit_mixture(n):
            # adds are the DVE-bound part; odd samples' adds run on GpSimd
            # (idle in this window) to halve the mixture wall time.
            add_eng = nc.vector if n % 2 == 0 else nc.gpsimd
            acc = mix_pool.tile([128, TG * V], BF16, tag="mix_acc", name="acc")
            tmp = mix_pool.tile([128, TG * V], BF16, tag="mix_tmp", name="tmp")
            nc.scalar.activation(
                acc[:], A_sb[:, 0, :], ACTF.Copy, scale=w_bcast[:, n * E:n * E + 1]
            )
            nc.vector.tensor_scalar(
                tmp[:], A_sb[:, 1, :], w_bcast[:, n * E + 1:n * E + 2], None, ALU.mult
            )
            add_eng.tensor_tensor(acc[:], acc[:], tmp[:], ALU.add)
            nc.scalar.activation(
                tmp[:], A_sb[:, 2, :], ACTF.Copy, scale=w_bcast[:, n * E + 2:n * E + 3]
            )
            add_eng.tensor_tensor(acc[:], acc[:], tmp[:], ALU.add)
            nc.vector.tensor_scalar(
                tmp[:], A_sb[:, 3, :], w_bcast[:, n * E + 3:n * E + 4], None, ALU.mult
            )
            add_eng.tensor_tensor(AS_sb[:, n, :], acc[:], tmp[:], ALU.add)

        def emit_conv_pair(pr):
            ot = out_pool.tile([128, T * V], BF16, tag="ot", name="ot")
            for g0, glen in ((0, 20), (20, 12)):
                # width padded to 512 so the row stride is bank-aligned
                pob = [
                    pc.tile([128, 512], FP32, tag=f"po{b}", name=f"po{b}",
                            bufs=2 if b < 2 else 1)
                    for b in range(4)
                ]
                for gi in range(glen):
                    g = g0 + gi
                    for b in range(4):
                        for j in range(2):
                            n = 2 * pr + j
                            nc.tensor.matmul(
                                pob[b][64 * j:64 * (j + 1),
                                       gi * V:(gi + 1) * V],
                                xcT_all[32 * b:32 * b + V, pr, g,
                                        64 * j:64 * (j + 1)],
                                AS_sb[32 * b:32 * b + V, n, g * V:(g + 1) * V],
                                start=True,
                                stop=True,
                                tile_position=(32 * b, 64 * j),
                            )
                width = glen * V
                for b in range(4):
                    dst = ot[:, (32 * b + g0) * V:(32 * b + g0) * V + width]
                    if b % 2 == 0:
                        nc.vector.tensor_copy(dst, pob[b][:, :width])
                    else:
                        nc.scalar.activation(dst, pob[b][:, :width], ACTF.Copy)
                dma_eng = nc.sync if pr % 2 == 0 else nc.scalar
                od = out[2 * pr:2 * pr + 2].rearrange("n c f -> (n c) f")
                dma_eng.dma_start(
                    od.rearrange("r (b q) -> r b q", b=4)[:, :, g0 * V:g0 * V + width],
                    ot.rearrange("r (b q) -> r b q", b=4)[:, :, g0 * V:g0 * V + width],
                )

        emit_mixture(0)
        emit_mixture(1)
        if phase == 4:
            emit_mixture(2)
            emit_mixture(3)
            ob = out.rearrange("n c f -> (n c) f")
            nc.sync.dma_start(ob[:C, :TG * V], AS_sb[:C, 0, :])
            return
        emit_conv_pair(0)
        emit_mixture(2)
        emit_mixture(3)
        emit_conv_pair(1)


_NC_CACHE = {}


def _get_nc(rep=None, phase=None):
    key = (rep or CFG["rep"], phase or CFG["phase"])
    if key not in _NC_CACHE:
        _NC_CACHE[key] = build(rep=key[0], phase=key[1])
    return _NC_CACHE[key]


def _to_bf16(a):
    """Round-to-nearest-even fp32 -> bf16, vectorized."""
    import ml_dtypes

    u = np.ascontiguousarray(a, dtype=np.float32).view(np.uint32)
    r = ((u + 0x7FFF + ((u >> 16) & 1)) >> 16).astype(np.uint16)
    return r.view(ml_dtypes.bfloat16)


def _to_fp8(a, shift):
    """fp32 -> TRN e4m3 (max 240) with power-of-2 scale 2^shift."""
    import ml_dtypes

    scaled = np.asarray(a, dtype=np.float32) * np.float32(2.0 ** shift)
    return np.clip(scaled, -240.0, 240.0).astype(ml_dtypes.float8_e4m3)


def _shard_inputs(x, W0, b0, W1, b1, W2, b2, A):
    x = np.ascontiguousarray(np.asarray(x, dtype=np.float32))
    W0 = np.ascontiguousarray(np.asarray(W0, dtype=np.float32))
    A = np.ascontiguousarray(np.asarray(A, dtype=np.float32)).reshape(E, T, V, V)
    xf = x.reshape(N, F)

    # A in padded layout: A4p[32b+v, e, g*V+w] = A[e, 32b+g, v, w]
    A4p = np.zeros((128, E, TG * V), dtype=np.float32)
    At = A.reshape(E, 4, TG, V, V)            # e b g v w
    for b in range(4):
        A4p[32 * b:32 * b + V, :, :] = (
            At[:, b].transpose(2, 0, 1, 3).reshape(V, E, TG * V)
        )
    A4p_bf = _to_bf16(A4p)

    blobf = np.concatenate([
        np.asarray(b0, np.float32).ravel(),
        np.asarray(W1, np.float32).ravel(),
        np.asarray(b1, np.float32).ravel(),
        np.asarray(W2, np.float32).ravel(),
        np.asarray(b2, np.float32).ravel(),
    ])

    in_maps = []
    for c in range(NCORES):
        # gating slice, pre-transposed to [128, KCH, N], fp8
        xg = xf[:, c * KS:(c + 1) * KS]                   # [N, KS]
        xgT = np.ascontiguousarray(
            xg.reshape(N, KCH, 128).transpose(2, 1, 0)    # [128, KCH, N]
        )
        blob8 = np.concatenate([
            _to_fp8(xgT, XSH).ravel(),
            _to_fp8(W0[c * KS:(c + 1) * KS], WSH).ravel(),
        ])

        # conv slice, pre-transposed/padded:
        # xcT[32b+v, pr, g, 64j+cc] = x[4c + 2pr + j, cc, 32b+g, v]
        xl = x[c * NLOC:(c + 1) * NLOC]                   # [4, C, T, V]
        xcT = np.zeros((128, NLOC // 2, TG, 2 * C), dtype=np.float32)
        xr = xl.reshape(NLOC // 2, 2, C, 4, TG, V)        # pr j cc b g v
        for b in range(4):
            blk = xr[:, :, :, b]                          # pr j cc g v
            xcT[32 * b:32 * b + V] = (
                blk.transpose(4, 0, 3, 1, 2).reshape(V, NLOC // 2, TG, 2 * C)
            )
        blobb = np.concatenate([_to_bf16(xcT).ravel(), A4p_bf.ravel()])

        in_maps.append({"blob8": blob8, "blobb": blobb, "blobf": blobf})
    return in_maps


def kernel(x, W0, b0, W1, b1, W2, b2, A):
    nc = _get_nc(rep=1, phase=5)
    in_maps = _shard_inputs(x, W0, b0, W1, b1, W2, b2, A)
    res = bass_utils.run_bass_kernel_spmd(nc, in_maps, core_ids=list(range(NCORES)))
    outs = [
        np.asarray(res.results[c]["out"], np.float32).reshape(NLOC, C, T, V)
        for c in range(NCORES)
    ]
    return np.concatenate(outs, axis=0)
